# revision 1
# baseline (speedup 1.0000x reference)
"""AWQ 4-bit quantized linear layer on 8 Trainium2 NeuronCores.

Computes out = x @ W.T + bias where W[o,i] = (q[o,i] - z[o,i//128]) * s[o,i//128],
q/z packed 8x int4 per int32.

Sharding: column-parallel (tensor-parallel on out_features). Each of the 8
cores gets qweight/qzeros/scales/bias rows [c*512, (c+1)*512) and the full
activation (shipped pre-transposed in bf16). Each core dequantizes its weight
shard on-chip — DVE nibble unpack (fused shift+mask tensor_scalar) + scale
multiply with 0-step group-broadcast APs, zero-point subtract on GPSIMD, PE
transposes into [K, N] layout with the PSUM->SBUF copies on the scalar
engine — then runs a bf16 matmul with fp32 PSUM accumulation. The dequant is
emitted in k-chunks interleaved with the first super-block's matmuls so the
(in-order) PE pipeline starts ~15us in; x loads alternate between the two
HWDGE rings. Host concatenates the 8 [B, 512] outputs along the feature axis.
"""

import os
import sys

for _p in ("/opt/trn_rl_repo", "/root/.axon_site/_ro/trn_rl_repo"):
    if os.path.isdir(_p) and _p not in sys.path:
        sys.path.insert(0, _p)

import numpy as np
import ml_dtypes

import concourse.bass as bass
import concourse.tile as tile
from concourse import bacc, mybir
from concourse.masks import make_identity

# Full-problem shapes (hardcoded; harness contract)
B_FULL = 8192
I_FULL = 4096
O_FULL = 4096
N_CORES = 8
GROUP = 128
PACK = 8

BF16 = mybir.dt.bfloat16
F32 = mybir.dt.float32
I32 = mybir.dt.int32


def build_bass(B, I, OS, m_super=512, repeat=1):
    """Build the per-core SPMD program.

    B: batch rows, I: in_features, OS: out_features per core.
    m_super: batch columns processed per super-block (multiple of 128).
    repeat: run the whole body N times (hardware For_i loop) - used only
    for timing measurements (wall-clock slope vs repeat).
    """
    KT = I // 128          # k-tiles (contraction)
    OT = OS // 128         # o-part-tiles in the shard
    NP = I // PACK         # packed int32 words per row
    NG = I // GROUP        # quantization groups
    NGP = (NG + PACK - 1) // PACK
    MSn = B // m_super     # m super-blocks
    M4 = m_super // 128    # 128-row m-tiles per super-block

    nc = bacc.Bacc("TRN2", target_bir_lowering=False)

    xT_d = nc.dram_tensor("xT", [I, B], BF16, kind="ExternalInput")
    qw_d = nc.dram_tensor("qw", [OS, NP], I32, kind="ExternalInput")
    qz_d = nc.dram_tensor("qz", [OS, NGP], I32, kind="ExternalInput")
    sc_d = nc.dram_tensor("sc", [OS, NG], F32, kind="ExternalInput")
    bi_d = nc.dram_tensor("bi", [OS], F32, kind="ExternalInput")
    out_d = nc.dram_tensor("out", [B, OS], F32, kind="ExternalOutput")

    with tile.TileContext(nc) as tc:
        with (
            tc.tile_pool(name="const", bufs=1) as const,
            tc.tile_pool(name="wt", bufs=1) as wtp,
            tc.tile_pool(name="dq", bufs=2) as dq,
            tc.tile_pool(name="xp", bufs=2) as xp,
            tc.tile_pool(name="ob", bufs=4) as ob,
            tc.tile_pool(name="ps", bufs=8, space="PSUM") as ps,
        ):
            rep_ctx = tc.For_i(0, repeat, 1) if repeat > 1 else None
            if rep_ctx is not None:
                rep_ctx.__enter__()
            ident = const.tile([128, 128], BF16)
            make_identity(nc, ident[:])

            # bias broadcast to [128, OS] (varies along free dim of out tiles)
            bias_bc = const.tile([128, OS], F32)
            bi_ap = bi_d[:]
            nc.gpsimd.dma_start(
                out=bias_bc[:],
                in_=bass.AP(tensor=bi_ap.tensor, offset=0, ap=[[0, 128], [1, OS]]),
            )

            # Dequantized weight, [k-tile partition(i), KT, OS] bf16, resident
            WT = wtp.tile([128, KT, OS], BF16)

            # ---- dequantization ----
            # Stage 1: per-o-tile constants (scales, zero*scale, expansions)
            qw_ts, s_fulls, zs_fulls = [], [], []
            for ot in range(OT):
                qw_t = dq.tile([128, NP], I32, name="qw_t", tag="qw_t", bufs=OT)
                nc.sync.dma_start(qw_t[:], qw_d[ot * 128:(ot + 1) * 128, :])
                s_t = dq.tile([128, NG], F32, name="s_t", tag="s_t", bufs=OT)
                nc.sync.dma_start(s_t[:], sc_d[ot * 128:(ot + 1) * 128, :])
                qz_t = dq.tile([128, NGP], I32)
                nc.sync.dma_start(qz_t[:], qz_d[ot * 128:(ot + 1) * 128, :])

                # unpack zero-points: z[o, g], g = 8*pc + j
                z_t = dq.tile([128, NG], I32)
                z_v = z_t.rearrange("p (pc j) -> p pc j", j=PACK)
                for j in range(PACK):
                    nc.vector.tensor_scalar(
                        out=z_v[:, :, j],
                        in0=qz_t[:],
                        scalar1=4 * j,
                        scalar2=0xF,
                        op0=mybir.AluOpType.logical_shift_right,
                        op1=mybir.AluOpType.bitwise_and,
                    )
                # int32 x f32 -> f32 (DVE converts inputs before the ALU)
                zs_t = dq.tile([128, NG], F32, name="zs_t", tag="zs_t", bufs=OT)
                nc.vector.tensor_mul(zs_t[:], z_t[:], s_t[:])
                qw_ts.append(qw_t)
                s_fulls.append(s_t)
                zs_fulls.append(zs_t)

            # xT viewed as [p, kt, b] so one DMA loads all k-tiles of a
            # super-block (amortizes HWDGE fixed cost)
            xT_v = xT_d.rearrange("(kt p) b -> p kt b", p=128)
            out_v = out_d.rearrange("(ms m4 p) o -> ms p m4 o", p=128, m4=M4)

            def load_x(ms):
                xtile = xp.tile([128, KT, m_super], BF16, name="xtile", tag="xtile")
                eng = nc.sync if ms % 2 == 0 else nc.scalar
                eng.dma_start(
                    xtile[:], xT_v[:, :, ms * m_super:(ms + 1) * m_super]
                )
                return xtile

            def mm_run(pss, xtile, m4, ks):
                # consecutive matmuls into the SAME psum bank (avoids
                # per-instruction psum bank cycling)
                for k in ks:
                    nc.tensor.matmul(
                        pss[m4][:],
                        xtile[:, k, m4 * 128:(m4 + 1) * 128],
                        WT[:, k, :],
                        start=(k == 0),
                        stop=(k == KT - 1),
                    )

            def evict(pss, ms):
                o_sb = ob.tile([128, M4, OS], F32, name="o_sb", tag="o_sb")
                for m4 in range(M4):
                    nc.vector.tensor_add(o_sb[:, m4, :], pss[m4][:], bias_bc[:])
                # store via the second HWDGE ring (Activation) to keep the
                # SP ring free for x loads
                nc.scalar.dma_start(out_v[ms], o_sb[:])

            # Stage 2: unpack + scale + transpose in chunks of k-tiles, with
            # the first super-block's matmuls interleaved chunk-by-chunk.
            # PE executes in program order, so transposes must alternate with
            # matmuls in emission order for the pipeline to start early.
            KCH = min(8, KT)           # k-tiles per chunk
            PCH = KCH * 16             # packed words per chunk
            xtile0 = load_x(0)
            pss0 = [ps.tile([128, OS], F32, name="acc", tag="acc")
                    for _ in range(M4)]
            for kc in range((KT + KCH - 1) // KCH):
                for ot in range(OT):
                    psl = slice(kc * PCH, (kc + 1) * PCH)
                    # per-group scale / zero*scale read with a 0-step inner
                    # dim (each group value repeated 16x along the free dim)
                    def bcast(t):
                        sl = t[:, kc * KCH:(kc + 1) * KCH]
                        return bass.AP(tensor=sl.tensor, offset=sl.offset,
                                       ap=[sl.ap[0], sl.ap[1], [0, 16]])
                    s_b = bcast(s_fulls[ot])
                    zs_b = bcast(zs_fulls[ot])
                    # W[o, 8p+j] = nib * s - z*s for p in this chunk
                    W_sb = dq.tile([128, PCH * PACK], BF16, name="W_sb", tag="W_sb")
                    W_v = W_sb.rearrange("p (pk j) -> p pk j", j=PACK)
                    for j in range(PACK):
                        nib = dq.tile([128, PCH], I32)
                        nc.vector.tensor_scalar(
                            out=nib[:],
                            in0=qw_ts[ot][:, psl],
                            scalar1=4 * j,
                            scalar2=0xF,
                            op0=mybir.AluOpType.logical_shift_right,
                            op1=mybir.AluOpType.bitwise_and,
                        )
                        nibf = dq.tile([128, PCH], F32)
                        nc.vector.tensor_tensor(
                            out=nibf.rearrange("p (g r) -> p g r", r=16),
                            in0=nib.rearrange("p (g r) -> p g r", r=16),
                            in1=s_b, op=mybir.AluOpType.mult)
                        nc.gpsimd.tensor_tensor(
                            out=W_v[:, :, j].rearrange("p (g r) -> p g r", r=16),
                            in0=nibf.rearrange("p (g r) -> p g r", r=16),
                            in1=zs_b, op=mybir.AluOpType.subtract)

                    # transpose [128 o, 128 i] blocks -> WT[i, k, o]
                    for kl in range(KCH):
                        k = kc * KCH + kl
                        tp = ps.tile([128, 128], BF16, name="acc", tag="acc")
                        nc.tensor.transpose(
                            tp[:], W_sb[:, kl * 128:(kl + 1) * 128], ident[:]
                        )
                        nc.scalar.copy(WT[:, k, ot * 128:(ot + 1) * 128], tp[:])
                # ms=0 matmuls for this chunk's k-tiles (8 consecutive
                # same-bank matmuls per m4)
                ks = [k for k in range(kc * KCH, min((kc + 1) * KCH, KT))]
                for m4 in range(M4):
                    mm_run(pss0, xtile0, m4, ks)
            evict(pss0, 0)

            # ---- remaining super-blocks ----
            for ms in range(1, MSn):
                xtile = load_x(ms)
                pss = [ps.tile([128, OS], F32, name="acc", tag="acc")
                       for _ in range(M4)]
                for m4 in range(M4):
                    mm_run(pss, xtile, m4, range(KT))
                evict(pss, ms)

            if rep_ctx is not None:
                rep_ctx.__exit__(None, None, None)

    nc.compile()
    return nc


_NC_CACHE = {}


def _get_nc(B, I, OS, repeat=1):
    key = (B, I, OS, repeat)
    if key not in _NC_CACHE:
        _NC_CACHE[key] = build_bass(B, I, OS, repeat=repeat)
    return _NC_CACHE[key]


def make_in_maps(x, qweight, qzeros, scales, bias, n_cores=N_CORES):
    O = qweight.shape[0]
    OS = O // n_cores
    xT = np.ascontiguousarray(x.T).astype(ml_dtypes.bfloat16)
    in_maps = []
    for c in range(n_cores):
        sl = slice(c * OS, (c + 1) * OS)
        in_maps.append({
            "xT": xT,
            "qw": np.ascontiguousarray(qweight[sl]),
            "qz": np.ascontiguousarray(qzeros[sl]),
            "sc": np.ascontiguousarray(scales[sl]),
            "bi": np.ascontiguousarray(bias[sl]),
        })
    return in_maps


def kernel(x, qweight, qzeros, scales, bias):
    from concourse.bass_utils import run_bass_kernel_spmd

    B, I = x.shape
    O = qweight.shape[0]
    OS = O // N_CORES
    nc = _get_nc(B, I, OS)
    in_maps = make_in_maps(x, qweight, qzeros, scales, bias)
    res = run_bass_kernel_spmd(nc, in_maps, core_ids=list(range(N_CORES)))
    out = np.concatenate([res.results[c]["out"] for c in range(N_CORES)], axis=1)
    return out.astype(np.float32)



# revision 4
# speedup vs baseline: 1.0050x; 1.0050x over previous
"""AWQ 4-bit quantized linear layer on 8 Trainium2 NeuronCores.

Computes out = x @ W.T + bias where W[o,i] = (q[o,i] - z[o,i//128]) * s[o,i//128],
q/z packed 8x int4 per int32.

Sharding: column-parallel (tensor-parallel on out_features). Each of the 8
cores gets the weight rows [c*512, (c+1)*512) and the full activation
(shipped pre-transposed in bf16).

v2 layout: qweight is host-repacked (pure nibble shuffle) into [in_features,
out_words] order so the on-chip unpack lands directly in the matmul's
[i-partition, k, o] layout -- no PE transposes and no PSUM->SBUF copies.
Scales and zero*scale are DMA-replicated across the 128 partitions (small),
and dequant is 3 dense DVE ops per nibble plane (shift+mask, *s, -z*s),
emitted in k-chunks interleaved with the first two super-blocks' matmuls so
the (in-order) PE pipeline saturates early. The k-tile size equals the AWQ
group size (128), so each k-tile sees a single scale column.
"""

import os
import sys

for _p in ("/opt/trn_rl_repo", "/root/.axon_site/_ro/trn_rl_repo"):
    if os.path.isdir(_p) and _p not in sys.path:
        sys.path.insert(0, _p)

import numpy as np
import ml_dtypes

import concourse.bass as bass
import concourse.tile as tile
from concourse import bacc, mybir

# Full-problem shapes (hardcoded; harness contract)
B_FULL = 8192
I_FULL = 4096
O_FULL = 4096
N_CORES = 8
GROUP = 128
PACK = 8

BF16 = mybir.dt.bfloat16
F32 = mybir.dt.float32
I32 = mybir.dt.int32


def build_bass(B, I, OS, m_super=512, repeat=1):
    """Build the per-core SPMD program.

    B: batch rows, I: in_features, OS: out_features per core.
    m_super: batch columns processed per super-block (multiple of 128).
    repeat: run the whole body N times (hardware For_i loop) - used only
    for timing measurements (wall-clock slope vs repeat).
    """
    KT = I // 128          # k-tiles (contraction); one AWQ group per k-tile
    NG = I // GROUP        # quantization groups == KT
    NW = OS // PACK        # packed words per i-row (o-direction packing)
    MSn = B // m_super     # m super-blocks
    M4 = m_super // 128    # 128-row m-tiles per super-block
    KCH = 8                # k-tiles dequantized per chunk
    NKC = KT // KCH

    nc = bacc.Bacc("TRN2", target_bir_lowering=False)

    xT_d = nc.dram_tensor("xT", [I, B], BF16, kind="ExternalInput")
    qw_d = nc.dram_tensor("qw", [I, NW], I32, kind="ExternalInput")
    sj_d = nc.dram_tensor("sj", [PACK, NG, NW], BF16, kind="ExternalInput")
    zj_d = nc.dram_tensor("zj", [PACK, NG, NW], BF16, kind="ExternalInput")
    bi_d = nc.dram_tensor("bi", [OS], F32, kind="ExternalInput")
    out_d = nc.dram_tensor("out", [B, OS], F32, kind="ExternalOutput")

    with tile.TileContext(nc) as tc:
        with (
            tc.tile_pool(name="const", bufs=1) as const,
            tc.tile_pool(name="wt", bufs=1) as wtp,
            tc.tile_pool(name="dq", bufs=2) as dq,
            tc.tile_pool(name="xp", bufs=2) as xp,
            tc.tile_pool(name="ob", bufs=2) as ob,
            tc.tile_pool(name="ps", bufs=8, space="PSUM") as ps,
        ):
            rep_ctx = tc.For_i(0, repeat, 1) if repeat > 1 else None
            if rep_ctx is not None:
                rep_ctx.__enter__()

            # bias broadcast to [128, OS] (varies along free dim of out tiles)
            bias_bc = const.tile([128, OS], F32)
            nc.gpsimd.dma_start(
                out=bias_bc[:],
                in_=bass.AP(tensor=bi_d[:].tensor, offset=0,
                            ap=[[0, 128], [1, OS]]),
            )

            # Dequantized weight, [i-partition, KT, OS] bf16, resident
            WT = wtp.tile([128, KT, OS], BF16)

            # packed weight, [i-partition, kt, word] (word w holds o = 64j+w)
            qw_sb = dq.tile([128, KT, NW], I32, name="qw_sb", tag="qw_sb",
                            bufs=1)
            qw_v = qw_d.rearrange("(kt p) w -> p kt w", p=128)
            nc.sync.dma_start(qw_sb[:], qw_v)

            # xT viewed as [p, kt, b] so one DMA loads all k-tiles of a
            # super-block (amortizes HWDGE fixed cost)
            xT_v = xT_d.rearrange("(kt p) b -> p kt b", p=128)
            out_v = out_d.rearrange("(ms m4 p) o -> ms p m4 o", p=128, m4=M4)

            def load_x(ms):
                xtile = xp.tile([128, KT, m_super], BF16, name="xtile",
                                tag="xtile")
                eng = nc.sync if ms % 2 == 0 else nc.scalar
                eng.dma_start(
                    xtile[:], xT_v[:, :, ms * m_super:(ms + 1) * m_super]
                )
                return xtile

            def mm_run(pss, xtile, m4, ks):
                # consecutive matmuls into the SAME psum bank (avoids
                # per-instruction psum bank cycling)
                for k in ks:
                    nc.tensor.matmul(
                        pss[m4][:],
                        xtile[:, k, m4 * 128:(m4 + 1) * 128],
                        WT[:, k, :],
                        start=(k == 0),
                        stop=(k == KT - 1),
                    )

            def evict(pss, ms):
                o_sb = ob.tile([128, M4, OS], F32, name="o_sb", tag="o_sb")
                for m4 in range(M4):
                    nc.vector.tensor_add(o_sb[:, m4, :], pss[m4][:], bias_bc[:])
                nc.scalar.dma_start(out_v[ms], o_sb[:])

            # ---- dequant interleaved with the first two super-blocks ----
            xtile0 = load_x(0)
            xtile1 = load_x(1)
            pss01 = [
                [ps.tile([128, OS], F32, name="acc", tag="acc")
                 for _ in range(M4)]
                for _ in range(2)
            ]
            for kc in range(NKC):
                ksl = slice(kc * KCH, (kc + 1) * KCH)
                # replicate this chunk's scale / zero*scale rows across the
                # 128 partitions (j-major so per-j slices are dense)
                s_bc = dq.tile([128, PACK, KCH, NW], BF16, name="s_bc",
                               tag="s_bc", bufs=2)
                zs_bc = dq.tile([128, PACK, KCH, NW], BF16, name="zs_bc",
                                tag="zs_bc", bufs=2)
                nc.gpsimd.dma_start(
                    out=s_bc[:],
                    in_=bass.AP(tensor=sj_d[:].tensor, offset=kc * KCH * NW,
                                ap=[[0, 128], [NG * NW, PACK], [1, KCH * NW]]),
                )
                nc.gpsimd.dma_start(
                    out=zs_bc[:],
                    in_=bass.AP(tensor=zj_d[:].tensor, offset=kc * KCH * NW,
                                ap=[[0, 128], [NG * NW, PACK], [1, KCH * NW]]),
                )
                for j in range(PACK):
                    # nib = (word >> 4j) & 0xF (bitVec ops cannot cast; the
                    # following mult converts int32 inputs before the ALU)
                    nib = dq.tile([128, KCH, NW], I32, name="nib", tag="nib")
                    nc.vector.tensor_scalar(
                        out=nib[:],
                        in0=qw_sb[:, ksl, :],
                        scalar1=4 * j,
                        scalar2=0xF,
                        op0=mybir.AluOpType.logical_shift_right,
                        op1=mybir.AluOpType.bitwise_and,
                    )
                    nibf = dq.tile([128, KCH, NW], BF16, name="nibf",
                                   tag="nibf")
                    nc.vector.tensor_tensor(
                        out=nibf[:], in0=nib[:], in1=s_bc[:, j],
                        op=mybir.AluOpType.mult)
                    # W[i, k, 64j+w] = nib*s - z*s
                    nc.vector.tensor_tensor(
                        out=WT[:, ksl, 64 * j:64 * j + 64],
                        in0=nibf[:], in1=zs_bc[:, j],
                        op=mybir.AluOpType.subtract)
                # this chunk's k-tiles for super-blocks 0 and 1 (keeps PE
                # fed while the next chunk dequantizes)
                ks = list(range(kc * KCH, (kc + 1) * KCH))
                for msi in range(2):
                    for m4 in range(M4):
                        mm_run(pss01[msi], (xtile0, xtile1)[msi], m4, ks)
            evict(pss01[0], 0)
            evict(pss01[1], 1)

            # ---- remaining super-blocks ----
            for ms in range(2, MSn):
                xtile = load_x(ms)
                pss = [ps.tile([128, OS], F32, name="acc", tag="acc")
                       for _ in range(M4)]
                for m4 in range(M4):
                    mm_run(pss, xtile, m4, range(KT))
                evict(pss, ms)

            if rep_ctx is not None:
                rep_ctx.__exit__(None, None, None)

    nc.compile()
    return nc


_NC_CACHE = {}


def _get_nc(B, I, OS, repeat=1):
    key = (B, I, OS, repeat)
    if key not in _NC_CACHE:
        _NC_CACHE[key] = build_bass(B, I, OS, repeat=repeat)
    return _NC_CACHE[key]


def _unpack_int4_np(packed):
    """[N, W] int32 -> [N, W*8] uint8 nibbles (low nibble first)."""
    u = packed.view(np.uint32)
    shifts = (np.arange(PACK, dtype=np.uint32) * 4)[None, None, :]
    vals = (u[:, :, None] >> shifts) & np.uint32(0xF)
    return vals.reshape(packed.shape[0], -1).astype(np.uint8)


def make_in_maps(x, qweight, qzeros, scales, bias, n_cores=N_CORES):
    O = qweight.shape[0]
    I = x.shape[1]
    OS = O // n_cores
    NW = OS // PACK
    NG = I // GROUP
    xT = np.ascontiguousarray(x.T).astype(ml_dtypes.bfloat16)
    q4 = _unpack_int4_np(qweight)                  # [O, I]
    z4 = _unpack_int4_np(qzeros)[:, :NG]           # [O, NG]
    zs = z4.astype(np.float32) * scales            # [O, NG]
    jshift = (np.arange(PACK, dtype=np.uint32) * 4)[:, None, None]
    in_maps = []
    for c in range(n_cores):
        sl = slice(c * OS, (c + 1) * OS)
        # repack nibbles o-major: word[i, w] holds o_local = 64*j + w
        t = q4[sl].reshape(PACK, NW, I).astype(np.uint32)   # [j, w, i]
        qwT = np.ascontiguousarray(
            (t << jshift).sum(axis=0, dtype=np.uint32).T).view(np.int32)
        # scale / zero*scale in [j, k, w] order matching the device layout
        sJ = np.ascontiguousarray(
            scales[sl].T.reshape(NG, PACK, NW).transpose(1, 0, 2)
        ).astype(ml_dtypes.bfloat16)
        zJ = np.ascontiguousarray(
            zs[sl].T.reshape(NG, PACK, NW).transpose(1, 0, 2)
        ).astype(ml_dtypes.bfloat16)
        in_maps.append({
            "xT": xT,
            "qw": qwT,
            "sj": sJ,
            "zj": zJ,
            "bi": np.ascontiguousarray(bias[sl]),
        })
    return in_maps


def kernel(x, qweight, qzeros, scales, bias):
    from concourse.bass_utils import run_bass_kernel_spmd

    B, I = x.shape
    O = qweight.shape[0]
    OS = O // N_CORES
    nc = _get_nc(B, I, OS)
    in_maps = make_in_maps(x, qweight, qzeros, scales, bias)
    res = run_bass_kernel_spmd(nc, in_maps, core_ids=list(range(N_CORES)))
    out = np.concatenate([res.results[c]["out"] for c in range(N_CORES)], axis=1)
    return out.astype(np.float32)


# revision 19
# speedup vs baseline: 1.0378x; 1.0327x over previous
"""AWQ 4-bit quantized linear layer on 8 Trainium2 NeuronCores.

Computes out = x @ W.T + bias where W[o,i] = (q[o,i] - z[o,i//128]) * s[o,i//128],
q/z packed 8x int4 per int32.

Sharding: column-parallel (tensor-parallel on out_features). Each of the 8
cores gets the weight rows [c*512, (c+1)*512) and the full activation
(shipped pre-transposed in bf16).

v2 layout: qweight is host-repacked (pure nibble shuffle) into [in_features,
out_words] order so the on-chip unpack lands directly in the matmul's
[i-partition, k, o] layout -- no PE transposes and no PSUM->SBUF copies.
Scales and zero*scale are DMA-replicated across the 128 partitions (small),
and dequant is 3 dense DVE ops per nibble plane (shift+mask, *s, -z*s),
emitted in k-chunks interleaved with the first two super-blocks' matmuls so
the (in-order) PE pipeline saturates early. The k-tile size equals the AWQ
group size (128), so each k-tile sees a single scale column.
"""

import os
import sys

for _p in ("/opt/trn_rl_repo", "/root/.axon_site/_ro/trn_rl_repo"):
    if os.path.isdir(_p) and _p not in sys.path:
        sys.path.insert(0, _p)

import numpy as np
import ml_dtypes

import concourse.bass as bass
import concourse.tile as tile
from concourse import bacc, mybir

# Full-problem shapes (hardcoded; harness contract)
B_FULL = 8192
I_FULL = 4096
O_FULL = 4096
N_CORES = 8
GROUP = 128
PACK = 8

BF16 = mybir.dt.bfloat16
F32 = mybir.dt.float32
I32 = mybir.dt.int32


def build_bass(B, I, OS, m_super=512, repeat=1,
               do_dequant=True, x_per_ms=True, do_stores=True, n_split=1):
    """Build the per-core SPMD program.

    B: batch rows, I: in_features, OS: out_features per core.
    m_super: batch columns processed per super-block (multiple of 128).
    repeat: run the whole body N times (hardware For_i loop) - used only
    for timing measurements (wall-clock slope vs repeat).
    do_dequant/x_per_ms/do_stores/n_split: ablation knobs for perf
    microbenchmarks (defaults = the real kernel).
    """
    KT = I // 128          # k-tiles (contraction); one AWQ group per k-tile
    NG = I // GROUP        # quantization groups == KT
    NW = OS // PACK        # packed words per i-row (o-direction packing)
    MSn = B // m_super     # m super-blocks
    M4 = m_super // 128    # 128-row m-tiles per super-block
    KCH = 8                # k-tiles dequantized per chunk
    NKC = KT // KCH

    nc = bacc.Bacc("TRN2", target_bir_lowering=False)

    xT_d = nc.dram_tensor("xT", [I, B], BF16, kind="ExternalInput")
    qw_d = nc.dram_tensor("qw", [I, NW], I32, kind="ExternalInput")
    # scale / zero*scale pre-broadcast across partitions, chunk-major so
    # each k-chunk is one contiguous HWDGE load
    sj_d = nc.dram_tensor("sj", [NKC, 128, PACK, KCH, NW], BF16,
                          kind="ExternalInput")
    zj_d = nc.dram_tensor("zj", [NKC, 128, PACK, KCH, NW], BF16,
                          kind="ExternalInput")
    bi_d = nc.dram_tensor("bi", [OS], F32, kind="ExternalInput")
    out_d = nc.dram_tensor("out", [B, OS], F32, kind="ExternalOutput")

    with tile.TileContext(nc) as tc:
        with (
            tc.tile_pool(name="const", bufs=1) as const,
            tc.tile_pool(name="wt", bufs=1) as wtp,
            tc.tile_pool(name="dq", bufs=2) as dq,
            tc.tile_pool(name="xp", bufs=3) as xp,
            tc.tile_pool(name="ob", bufs=2) as ob,
            tc.tile_pool(name="ps", bufs=8, space="PSUM") as ps,
        ):
            rep_ctx = tc.For_i(0, repeat, 1) if repeat > 1 else None
            if rep_ctx is not None:
                rep_ctx.__enter__()

            # bias broadcast to [128, OS] (varies along free dim of out tiles)
            bias_bc = const.tile([128, OS], F32)
            nc.gpsimd.dma_start(
                out=bias_bc[:],
                in_=bass.AP(tensor=bi_d[:].tensor, offset=0,
                            ap=[[0, 128], [1, OS]]),
            )

            # Dequantized weight, [i-partition, KT, OS] bf16, resident
            WT = wtp.tile([128, KT, OS], BF16)

            # packed weight, [i-partition, kt, word] (word w holds o = 64j+w)
            qw_sb = dq.tile([128, KT, NW], I32, name="qw_sb", tag="qw_sb",
                            bufs=1)
            qw_v = qw_d.rearrange("(kt p) w -> p kt w", p=128)
            nc.sync.dma_start(qw_sb[:], qw_v)

            # xT viewed as [p, kt, b] so one DMA loads all k-tiles of a
            # super-block (amortizes HWDGE fixed cost)
            xT_v = xT_d.rearrange("(kt p) b -> p kt b", p=128)
            out_v = out_d.rearrange("(ms m4 p) o -> ms p m4 o", p=128, m4=M4)

            def load_x(ms):
                xtile = xp.tile([128, KT, m_super], BF16, name="xtile",
                                tag="xtile")
                eng = nc.sync if ms % 2 == 0 else nc.scalar
                eng.dma_start(
                    xtile[:], xT_v[:, :, ms * m_super:(ms + 1) * m_super]
                )
                return xtile

            NSP = OS // n_split

            def mm_run(pss, xtile, m4, ks):
                # consecutive matmuls into the SAME psum bank (avoids
                # per-instruction psum bank cycling)
                for sp in range(n_split):
                    for k in ks:
                        nc.tensor.matmul(
                            pss[m4 * n_split + sp][:],
                            xtile[:, k, m4 * 128:(m4 + 1) * 128],
                            WT[:, k, sp * NSP:(sp + 1) * NSP],
                            start=(k == 0),
                            stop=(k == KT - 1),
                        )

            def evict(pss, ms, force_store=False):
                o_sb = ob.tile([128, M4, OS], F32, name="o_sb", tag="o_sb")
                for m4 in range(M4):
                    for sp in range(n_split):
                        nc.vector.tensor_add(
                            o_sb[:, m4, sp * NSP:(sp + 1) * NSP],
                            pss[m4 * n_split + sp][:],
                            bias_bc[:, sp * NSP:(sp + 1) * NSP])
                if do_stores or force_store:
                    nc.scalar.dma_start(out_v[ms], o_sb[:])

            # ---- dequant interleaved with the first two super-blocks ----
            n_inter = 2 if n_split == 1 else 1   # super-blocks in flight
            xtile0 = load_x(0)
            xtile1 = (load_x(1) if x_per_ms else xtile0) if n_inter == 2 \
                else xtile0
            pss01 = [
                [ps.tile([128, NSP], F32, name="acc", tag="acc")
                 for _ in range(M4 * n_split)]
                for _ in range(n_inter)
            ]
            for kc in range(NKC):
                ksl = slice(kc * KCH, (kc + 1) * KCH)
                if do_dequant:
                    # this chunk's pre-broadcast scale / zero*scale
                    # (contiguous 2MB loads on the Activation ring)
                    s_bc = dq.tile([128, PACK, KCH, NW], BF16, name="s_bc",
                                   tag="s_bc", bufs=2)
                    zs_bc = dq.tile([128, PACK, KCH, NW], BF16, name="zs_bc",
                                    tag="zs_bc", bufs=2)
                    # SWDGE queue: contiguous loads, can't head-of-line
                    # block the x loads / stores on the HWDGE rings
                    nc.gpsimd.dma_start(s_bc[:], sj_d[kc])
                    nc.gpsimd.dma_start(zs_bc[:], zj_d[kc])
                    for j in range(PACK):
                        # nib = (word >> 4j) & 0xF (bitVec ops cannot cast;
                        # the mult converts int32 inputs before the ALU)
                        nib = dq.tile([128, KCH, NW], I32, name="nib",
                                      tag="nib")
                        nc.vector.tensor_scalar(
                            out=nib[:],
                            in0=qw_sb[:, ksl, :],
                            scalar1=4 * j,
                            scalar2=0xF,
                            op0=mybir.AluOpType.logical_shift_right,
                            op1=mybir.AluOpType.bitwise_and,
                        )
                        nibf = dq.tile([128, KCH, NW], BF16, name="nibf",
                                       tag="nibf")
                        nc.vector.tensor_tensor(
                            out=nibf[:], in0=nib[:], in1=s_bc[:, j],
                            op=mybir.AluOpType.mult)
                        # W[i, k, 64j+w] = nib*s - z*s
                        nc.vector.tensor_tensor(
                            out=WT[:, ksl, 64 * j:64 * j + 64],
                            in0=nibf[:], in1=zs_bc[:, j],
                            op=mybir.AluOpType.subtract)
                else:
                    # timing ablation: fill the chunk with finite garbage
                    nc.vector.memset(WT[:, ksl, :], 0.5)
                # this chunk's k-tiles for super-blocks 0 and 1 (keeps PE
                # fed while the next chunk dequantizes)
                ks = list(range(kc * KCH, (kc + 1) * KCH))
                for msi in range(n_inter):
                    for m4 in range(M4):
                        mm_run(pss01[msi], (xtile0, xtile1)[msi], m4, ks)
            for msi in range(n_inter):
                evict(pss01[msi], msi)

            # ---- remaining super-blocks ----
            for ms in range(n_inter, MSn):
                xtile = load_x(ms) if x_per_ms else xtile0
                pss = [ps.tile([128, NSP], F32, name="acc", tag="acc")
                       for _ in range(M4 * n_split)]
                for m4 in range(M4):
                    mm_run(pss, xtile, m4, range(KT))
                evict(pss, ms, force_store=(ms == MSn - 1))

            if rep_ctx is not None:
                rep_ctx.__exit__(None, None, None)

    nc.compile()
    return nc


_NC_CACHE = {}


def _get_nc(B, I, OS, repeat=1):
    key = (B, I, OS, repeat)
    if key not in _NC_CACHE:
        _NC_CACHE[key] = build_bass(B, I, OS, repeat=repeat)
    return _NC_CACHE[key]


def _unpack_int4_np(packed):
    """[N, W] int32 -> [N, W*8] uint8 nibbles (low nibble first)."""
    u = packed.view(np.uint32)
    shifts = (np.arange(PACK, dtype=np.uint32) * 4)[None, None, :]
    vals = (u[:, :, None] >> shifts) & np.uint32(0xF)
    return vals.reshape(packed.shape[0], -1).astype(np.uint8)


def make_in_maps(x, qweight, qzeros, scales, bias, n_cores=N_CORES):
    O = qweight.shape[0]
    I = x.shape[1]
    OS = O // n_cores
    NW = OS // PACK
    NG = I // GROUP
    xT = np.ascontiguousarray(x.T).astype(ml_dtypes.bfloat16)
    q4 = _unpack_int4_np(qweight)                  # [O, I]
    z4 = _unpack_int4_np(qzeros)[:, :NG]           # [O, NG]
    zs = z4.astype(np.float32) * scales            # [O, NG]
    jshift = (np.arange(PACK, dtype=np.uint32) * 4)[:, None, None]
    in_maps = []
    KCH = 8
    NKC = NG // KCH

    def chunk_bcast(v):
        # [OS, NG] -> pre-broadcast [NKC, 128, PACK, KCH, NW] bf16
        vj = v.T.reshape(NKC, KCH, PACK, NW).transpose(0, 2, 1, 3)
        return np.ascontiguousarray(
            np.broadcast_to(vj[:, None], (NKC, 128, PACK, KCH, NW))
        ).astype(ml_dtypes.bfloat16)

    for c in range(n_cores):
        sl = slice(c * OS, (c + 1) * OS)
        # repack nibbles o-major: word[i, w] holds o_local = 64*j + w
        t = q4[sl].reshape(PACK, NW, I).astype(np.uint32)   # [j, w, i]
        qwT = np.ascontiguousarray(
            (t << jshift).sum(axis=0, dtype=np.uint32).T).view(np.int32)
        in_maps.append({
            "xT": xT,
            "qw": qwT,
            "sj": chunk_bcast(scales[sl]),
            "zj": chunk_bcast(zs[sl]),
            "bi": np.ascontiguousarray(bias[sl]),
        })
    return in_maps


def kernel(x, qweight, qzeros, scales, bias):
    from concourse.bass_utils import run_bass_kernel_spmd

    B, I = x.shape
    O = qweight.shape[0]
    OS = O // N_CORES
    nc = _get_nc(B, I, OS)
    in_maps = make_in_maps(x, qweight, qzeros, scales, bias)
    res = run_bass_kernel_spmd(nc, in_maps, core_ids=list(range(N_CORES)))
    out = np.concatenate([res.results[c]["out"] for c in range(N_CORES)], axis=1)
    return out.astype(np.float32)
